# revision 50
# baseline (speedup 1.0000x reference)
"""EucNormLoss Trainium2 kernel (8-core SPMD).

loss = mean_i( sum_j d(i,j)*[l_i==l_j] / #{j: l_j==l_i} ),
d(i,j) = sqrt(relu(2 - 2*fn_i.fn_j)) on L2-normalized rows.

Only same-class pairs matter, so the host sorts rows by class (pure data
movement), pads each class to a fixed `slot` of rows, and deals an equal
number of class slots to each core.  Per slot-pair, PSUM accumulates
  psum = g - m_j*m_i - Dc*delta_ij
via bf16 matmuls (Gram + rank-1 validity mask + diagonal knockout), where
Dc = 2^-9 keeps -psum >= 0 everywhere: valid pairs give 1-g >= ~0.4, pad
pairs give exactly 0, diagonals give Dc + (1 - g_ii) > 0.  ACT then takes
sqrt(-psum) straight out of PSUM (no relu pass), and a bf16 4x-mode
tensor_scalar row-reduces each slot with weight 1/(n_c*N) via accum_out.
The known diagonal mass slot*sqrt(Dc)*sum_c 1/(n_c*N) is subtracted on
the host (its data-dependent residual is O(1e-6) relative).

Normalization happens on-device: per-row 1/max(||x||,1e-12) is folded
into the transpose as featT_tile = nat_tile.T @ diag(rinv), one fp32
matmul per 128-row tile; the PSUM->SBUF copy casts to bf16.

Structure is chunked so everything pipelines: 7 input-DMA chunks (issued
from both SP and ACT queues), per-chunk sumsq/norms/diag, per-quad featT
tiles, per-pair Gram PSUM.  Separate SBUF tiles per stage avoid WAR waits
(Tile tracks deps per tile; most ISA structs encode a single sync wait,
matmuls get a second via Bacc's move_matmul_waits_to_ldweights).
"""

import sys

import numpy as np

for _p in ("/opt/trn_rl_repo",):
    if _p not in sys.path:
        sys.path.insert(0, _p)

import ml_dtypes
from contextlib import ExitStack

import concourse.bass as bass
import concourse.bacc as bacc
import concourse.tile as tile
from concourse import mybir
from concourse.bass_utils import run_bass_kernel_spmd
from concourse.masks import make_identity

N_CORES = 8
P = 128          # partitions / feature dim
DC = 2.0 ** -6   # diagonal knockout; must exceed bf16 rounding of g_ii (half-ulp 2^-8)

F32 = mybir.dt.float32
BF16 = mybir.dt.bfloat16


def _bcast_rows(ap: bass.AP, n: int) -> bass.AP:
    """Broadcast a [1, ...] AP across n partitions (partition step 0)."""
    return bass.AP(tensor=ap.tensor, offset=ap.offset,
                   ap=[[0, n]] + list(ap.ap[1:]))


def _build_program(nslots: int, slot: int):
    rows = nslots * slot
    nt = rows // P               # 128-row tiles
    tps = slot // P              # tiles per slot
    spq = 2                      # slots per featT tile (aligns with pairs/chunks)
    nquads = -(-nslots // spq)
    npairs = -(-nslots // 2)

    nc = bacc.Bacc(None, target_bir_lowering=False)
    feat_d = nc.declare_dram_parameter("feat", [rows, P], F32, isOutput=False)
    mpos_d = nc.declare_dram_parameter("mpos", [1, rows], BF16, isOutput=False)
    mneg_d = nc.declare_dram_parameter("mneg", [1, rows], BF16, isOutput=False)
    wrow_d = nc.declare_dram_parameter("wrow", [1, nslots], F32, isOutput=False)
    out_d = nc.declare_dram_parameter("out", [1, 1], F32, isOutput=True)

    with ExitStack() as ctx:
        tc = ctx.enter_context(tile.TileContext(nc))
        consts = ctx.enter_context(tc.tile_pool(name="consts", bufs=1))
        singles = ctx.enter_context(tc.tile_pool(name="singles", bufs=1))
        ptp = ctx.enter_context(tc.tile_pool(name="ptp", bufs=2, space="PSUM"))
        gp = ctx.enter_context(tc.tile_pool(name="gp", bufs=2, space="PSUM"))
        csp = ctx.enter_context(tc.tile_pool(name="csp", bufs=2, space="PSUM"))

        # consts: ident (1.0 I), idn3 = [0 | -Dc*I | 0] for diag knockout
        ident = consts.tile([P, P], BF16)
        make_identity(nc, ident)
        idn3 = consts.tile([P, 3 * P], BF16)
        nc.gpsimd.memset(idn3, 0.0)
        nc.gpsimd.affine_select(
            out=idn3[:, P : 2 * P], in_=idn3[:, P : 2 * P],
            compare_op=mybir.AluOpType.not_equal,
            fill=-DC, base=0, pattern=[[-1, P]], channel_multiplier=1,
        )
        ones = consts.tile([P, 1], F32)
        nc.vector.memset(ones, 1.0)
        onesb = consts.tile([P, 1], BF16)
        nc.vector.memset(onesb, 1.0)
        # tiny bias inside sqrt(ss): pad rows get norm 1e-10 instead of 0,
        # so the reciprocal stays finite without a separate max() op
        nepsb = consts.tile([P, 1], F32)
        nc.vector.memset(nepsb, 1e-20)

        mpos = singles.tile([1, rows], BF16)
        mneg = singles.tile([1, rows], BF16)
        wb = singles.tile([P, nslots], F32)
        nc.sync.dma_start(out=mpos, in_=mpos_d[:, :])
        nc.sync.dma_start(out=mneg, in_=mneg_d[:, :])
        nc.sync.dma_start(out=wb, in_=_bcast_rows(wrow_d[:, :], P))

        # ---- chunked load + per-row sumsq + norms + diag(rinv) ----
        chunk = 4                                    # tiles per chunk
        cbounds = list(range(0, nt, chunk)) + [nt]
        nch = len(cbounds) - 1
        nats, diags, rinvs = [], [], []
        for c in range(nch):
            t0, t1 = cbounds[c], cbounds[c + 1]
            w = (t1 - t0) * P
            natc = singles.tile([P, w], F32, tag=f"nat{c}")
            src = feat_d[t0 * P : t1 * P, :]
            src3 = bass.AP(tensor=src.tensor, offset=src.offset,
                           ap=[[P, P], [P * P, t1 - t0], [1, P]])
            eng = nc.sync if c % 2 == 0 else nc.scalar
            eng.dma_start(out=natc[:, :].rearrange("p (t d) -> p t d", d=P),
                          in_=src3)
            sq = singles.tile([P, w], F32, tag=f"sq{c}")
            if c % 2 == 0:
                nc.gpsimd.tensor_mul(sq, natc, natc)
            else:
                nc.vector.tensor_mul(sq, natc, natc)
            ss = singles.tile([P, t1 - t0], F32, tag=f"ss{c}")
            nc.vector.tensor_reduce(
                ss, sq[:, :].rearrange("p (t d) -> p t d", d=P),
                axis=mybir.AxisListType.X, op=mybir.AluOpType.add,
            )
            nrm = singles.tile([P, t1 - t0], F32, tag=f"nr{c}")
            nc.scalar.activation(nrm, ss, mybir.ActivationFunctionType.Sqrt,
                                 bias=nepsb[:, 0:1])
            rinv = singles.tile([P, t1 - t0], F32, tag=f"ri{c}")
            nc.vector.reciprocal(rinv, nrm)
            dg = singles.tile([P, w], F32, tag=f"dg{c}")
            rb = bass.AP(tensor=rinv[:, :].tensor, offset=rinv[:, :].offset,
                         ap=list(rinv[:, :].ap) + [[0, P]])
            nc.gpsimd.affine_select(
                out=dg[:, :].rearrange("p (t d) -> p t d", d=P), in_=rb,
                compare_op=mybir.AluOpType.is_equal, fill=0.0, base=0,
                pattern=[[0, t1 - t0], [-1, P]], channel_multiplier=1,
            )
            nats.append((t0, natc))
            diags.append(dg)
            rinvs.append(rinv)

        def nat_tile(t):
            c = t // chunk
            t0, natc = nats[c]
            return natc[:, (t - t0) * P : (t - t0 + 1) * P]

        def diag_tile(t):
            c = t // chunk
            t0, _ = nats[c]
            return diags[c][:, (t - t0) * P : (t - t0 + 1) * P]

        # ---- normalize+transpose into per-quad bf16 featT tiles ----
        fts = []
        for q in range(nquads):
            s0, s1 = q * spq, min((q + 1) * spq, nslots)
            wq = (s1 - s0) * slot
            pt = ptp.tile([P, wq], F32, tag="pt")
            for k in range(wq // P):
                t = s0 * tps + k
                nc.tensor.matmul(pt[:, k * P : (k + 1) * P],
                                 nat_tile(t), diag_tile(t),
                                 start=True, stop=True)
            ft = singles.tile([P, wq], BF16, tag=f"ft{q}")
            nc.vector.tensor_copy(ft, pt)
            fts.append(ft)

        def ft_slice(s, a, b):
            return fts[s // spq][:, (s % spq) * slot + a : (s % spq) * slot + b]

        # ---- per-pair Gram PSUM + direct sqrt + PE column-sum reduce ----
        # all slots accumulate weighted column sums into one [1, slot] PSUM
        csall = csp.tile([1, slot], F32)
        for p in range(npairs):
            s0, s1 = 2 * p, min(2 * p + 2, nslots)
            wp = (s1 - s0) * tps * slot
            gt = gp.tile([P, wp], F32, tag="gt")
            # per region: diag knockout (const operands, absorbs the PSUM WAR
            # wait on its first use) -> mask -> Gram, closing each
            # accumulation group before the next opens
            for s in range(s0, s1):
                for h in range(tps):
                    reg = (s - s0) * tps * slot + h * slot
                    off = (tps - 1 - h) * P
                    jc = slice(s * slot + h * P, s * slot + (h + 1) * P)
                    ic = slice(s * slot, (s + 1) * slot)
                    nc.tensor.matmul(gt[:, reg : reg + slot],
                                     ident, idn3[:, off : off + slot],
                                     start=True, stop=False)
                    nc.tensor.matmul(gt[:, reg : reg + slot],
                                     mneg[:, jc], mpos[:, ic],
                                     start=False, stop=False)
                    nc.tensor.matmul(gt[:, reg : reg + slot],
                                     ft_slice(s, h * P, (h + 1) * P),
                                     ft_slice(s, 0, slot),
                                     start=False, stop=True)
            dq = singles.tile([P, wp], F32, tag=f"dq{p}")
            nc.scalar.activation(dq, gt, mybir.ActivationFunctionType.Sqrt,
                                 scale=-2.0)
            for s in range(s0, s1):
                o = (s - s0) * tps * slot
                # dist tiles are symmetric per slot: weighted column sums
                # over both j-halves accumulate the masked, weighted row
                # sums of every slot into one [1, slot] PSUM vector
                for h in range(tps):
                    nc.tensor.matmul(
                        csall, wb[:, s : s + 1],
                        dq[:, o + h * slot : o + (h + 1) * slot],
                        start=(s == 0 and h == 0),
                        stop=(s == nslots - 1 and h == tps - 1),
                    )

        # ---- core partial = sum(csall) -> DRAM ----
        partial = singles.tile([1, 1], F32)
        nc.vector.tensor_reduce(
            partial, csall, axis=mybir.AxisListType.X, op=mybir.AluOpType.add,
        )
        nc.sync.dma_start(out=out_d[:, :], in_=partial)

    nc.compile()
    return nc


def _shard_inputs(features: np.ndarray, labels: np.ndarray):
    """Sort rows by class, pad each class to a slot, deal slots to cores."""
    n = features.shape[0]
    classes, counts = np.unique(labels, return_counts=True)
    c = len(classes)
    nslots = -(-c // N_CORES)
    slot = max(256, -(-int(counts.max()) // P) * P)
    rows = nslots * slot

    order = np.argsort(labels, kind="stable")
    bounds = np.concatenate([[0], np.cumsum(counts)])

    in_maps = []
    for core in range(N_CORES):
        feat = np.zeros((rows, P), np.float32)
        mpos = np.zeros((1, rows), ml_dtypes.bfloat16)
        wrow = np.zeros((1, nslots), np.float32)
        for k in range(nslots):
            g = core * nslots + k
            if g >= c:
                continue
            cnt = int(counts[g])
            rows_g = order[bounds[g] : bounds[g + 1]]
            feat[k * slot : k * slot + cnt] = features[rows_g]
            mpos[0, k * slot : k * slot + cnt] = 1.0
            wrow[0, k] = 1.0 / (cnt * n)
        in_maps.append(
            {"feat": feat, "mpos": mpos, "mneg": -mpos, "wrow": wrow}
        )
    # host-side correction for the diagonal knockout mass
    corr = slot * np.sqrt(2.0 * DC) * float((1.0 / (counts.astype(np.float64) * n)).sum())
    return in_maps, nslots, slot, corr


def _run(features, labels, **spmd_kwargs):
    features = np.asarray(features, np.float32)
    labels = np.asarray(labels).reshape(-1)
    in_maps, nslots, slot, corr = _shard_inputs(features, labels)
    nc = _build_program(nslots, slot)
    res = run_bass_kernel_spmd(nc, in_maps, core_ids=list(range(N_CORES)),
                               **spmd_kwargs)
    total = 0.0
    for r in res.results:
        total += float(r["out"].reshape(-1)[0])
    return np.float32(total - corr), res


def kernel(features, labels):
    out, _ = _run(features, labels)
    return out


# revision 51
# speedup vs baseline: 1.1551x; 1.1551x over previous
"""EucNormLoss Trainium2 kernel (8-core SPMD).

loss = mean_i( sum_j d(i,j)*[l_i==l_j] / #{j: l_j==l_i} ),
d(i,j) = sqrt(relu(2 - 2*fn_i.fn_j)) on L2-normalized rows.

Only same-class pairs matter, so the host sorts rows by class (pure data
movement), pads each class to a fixed `slot` of rows, and deals an equal
number of class slots to each core.  Per slot-pair, PSUM accumulates
  psum = g - m_j*m_i - Dc*delta_ij
via bf16 matmuls (Gram + rank-1 validity mask + diagonal knockout), where
Dc = 2^-9 keeps -psum >= 0 everywhere: valid pairs give 1-g >= ~0.4, pad
pairs give exactly 0, diagonals give Dc + (1 - g_ii) > 0.  ACT then takes
sqrt(-psum) straight out of PSUM (no relu pass), and a bf16 4x-mode
tensor_scalar row-reduces each slot with weight 1/(n_c*N) via accum_out.
The known diagonal mass slot*sqrt(Dc)*sum_c 1/(n_c*N) is subtracted on
the host (its data-dependent residual is O(1e-6) relative).

Normalization happens on-device: per-row 1/max(||x||,1e-12) is folded
into the transpose as featT_tile = nat_tile.T @ diag(rinv), one fp32
matmul per 128-row tile; the PSUM->SBUF copy casts to bf16.

Structure is chunked so everything pipelines: 7 input-DMA chunks (issued
from both SP and ACT queues), per-chunk sumsq/norms/diag, per-quad featT
tiles, per-pair Gram PSUM.  Separate SBUF tiles per stage avoid WAR waits
(Tile tracks deps per tile; most ISA structs encode a single sync wait,
matmuls get a second via Bacc's move_matmul_waits_to_ldweights).
"""

import sys

import numpy as np

for _p in ("/opt/trn_rl_repo",):
    if _p not in sys.path:
        sys.path.insert(0, _p)

import ml_dtypes
from contextlib import ExitStack

import concourse.bass as bass
import concourse.bacc as bacc
import concourse.tile as tile
from concourse import mybir
from concourse.bass_utils import run_bass_kernel_spmd
from concourse.masks import make_identity

N_CORES = 8
P = 128          # partitions / feature dim
DC = 2.0 ** -6   # diagonal knockout; must exceed bf16 rounding of g_ii (half-ulp 2^-8)

F32 = mybir.dt.float32
BF16 = mybir.dt.bfloat16


def _bcast_rows(ap: bass.AP, n: int) -> bass.AP:
    """Broadcast a [1, ...] AP across n partitions (partition step 0)."""
    return bass.AP(tensor=ap.tensor, offset=ap.offset,
                   ap=[[0, n]] + list(ap.ap[1:]))


def _build_program(nslots: int, slot: int):
    rows = nslots * slot
    nt = rows // P               # 128-row tiles
    tps = slot // P              # tiles per slot
    spq = 2                      # slots per featT tile (aligns with pairs/chunks)
    nquads = -(-nslots // spq)
    npairs = -(-nslots // 2)

    nc = bacc.Bacc(None, target_bir_lowering=False)
    feat_d = nc.declare_dram_parameter("feat", [rows, P], F32, isOutput=False)
    mpos_d = nc.declare_dram_parameter("mpos", [1, rows], BF16, isOutput=False)
    mneg_d = nc.declare_dram_parameter("mneg", [1, rows], BF16, isOutput=False)
    wrow_d = nc.declare_dram_parameter("wrow", [1, nslots], F32, isOutput=False)
    out_d = nc.declare_dram_parameter("out", [1, 1], F32, isOutput=True)

    with ExitStack() as ctx:
        tc = ctx.enter_context(tile.TileContext(nc))
        consts = ctx.enter_context(tc.tile_pool(name="consts", bufs=1))
        singles = ctx.enter_context(tc.tile_pool(name="singles", bufs=1))
        ptp = ctx.enter_context(tc.tile_pool(name="ptp", bufs=2, space="PSUM"))
        gp = ctx.enter_context(tc.tile_pool(name="gp", bufs=2, space="PSUM"))
        csp = ctx.enter_context(tc.tile_pool(name="csp", bufs=2, space="PSUM"))

        # consts: ident (1.0 I), idn3 = [0 | -Dc*I | 0] for diag knockout
        ident = consts.tile([P, P], BF16)
        make_identity(nc, ident)
        idn3 = consts.tile([P, 3 * P], BF16)
        nc.gpsimd.memset(idn3, 0.0)
        nc.gpsimd.affine_select(
            out=idn3[:, P : 2 * P], in_=idn3[:, P : 2 * P],
            compare_op=mybir.AluOpType.not_equal,
            fill=-DC, base=0, pattern=[[-1, P]], channel_multiplier=1,
        )
        ones = consts.tile([P, 1], F32)
        nc.vector.memset(ones, 1.0)
        onesb = consts.tile([P, 1], BF16)
        nc.vector.memset(onesb, 1.0)
        # tiny bias inside sqrt(ss): pad rows get norm 1e-10 instead of 0,
        # so the reciprocal stays finite without a separate max() op
        nepsb = consts.tile([P, 1], F32)
        nc.vector.memset(nepsb, 1e-20)

        mpos = singles.tile([1, rows], BF16)
        mneg = singles.tile([1, rows], BF16)
        wb = singles.tile([P, nslots], F32)
        nc.sync.dma_start(out=mpos, in_=mpos_d[:, :])
        nc.sync.dma_start(out=mneg, in_=mneg_d[:, :])
        nc.sync.dma_start(out=wb, in_=_bcast_rows(wrow_d[:, :], P))

        # ---- chunked load + per-row sumsq + norms + diag(rinv) ----
        chunk = 4                                    # tiles per chunk
        cbounds = list(range(0, nt, chunk)) + [nt]
        nch = len(cbounds) - 1
        nats, diags, rinvs = [], [], []
        for c in range(nch):
            t0, t1 = cbounds[c], cbounds[c + 1]
            w = (t1 - t0) * P
            natc = singles.tile([P, w], F32, tag=f"nat{c}")
            src = feat_d[t0 * P : t1 * P, :]
            src3 = bass.AP(tensor=src.tensor, offset=src.offset,
                           ap=[[P, P], [P * P, t1 - t0], [1, P]])
            eng = nc.sync if c % 2 == 0 else nc.scalar
            eng.dma_start(out=natc[:, :].rearrange("p (t d) -> p t d", d=P),
                          in_=src3)
            sq = singles.tile([P, w], F32, tag=f"sq{c}")
            if c % 2 == 0:
                nc.gpsimd.tensor_mul(sq, natc, natc)
            else:
                nc.vector.tensor_mul(sq, natc, natc)
            ss = singles.tile([P, t1 - t0], F32, tag=f"ss{c}")
            nc.vector.tensor_reduce(
                ss, sq[:, :].rearrange("p (t d) -> p t d", d=P),
                axis=mybir.AxisListType.X, op=mybir.AluOpType.add,
            )
            nrm = singles.tile([P, t1 - t0], F32, tag=f"nr{c}")
            nc.scalar.activation(nrm, ss, mybir.ActivationFunctionType.Sqrt,
                                 bias=nepsb[:, 0:1])
            rinv = singles.tile([P, t1 - t0], F32, tag=f"ri{c}")
            nc.vector.reciprocal(rinv, nrm)
            dg = singles.tile([P, w], F32, tag=f"dg{c}")
            rb = bass.AP(tensor=rinv[:, :].tensor, offset=rinv[:, :].offset,
                         ap=list(rinv[:, :].ap) + [[0, P]])
            nc.gpsimd.affine_select(
                out=dg[:, :].rearrange("p (t d) -> p t d", d=P), in_=rb,
                compare_op=mybir.AluOpType.is_equal, fill=0.0, base=0,
                pattern=[[0, t1 - t0], [-1, P]], channel_multiplier=1,
            )
            nats.append((t0, natc))
            diags.append(dg)
            rinvs.append(rinv)

        def nat_tile(t):
            c = t // chunk
            t0, natc = nats[c]
            return natc[:, (t - t0) * P : (t - t0 + 1) * P]

        def diag_tile(t):
            c = t // chunk
            t0, _ = nats[c]
            return diags[c][:, (t - t0) * P : (t - t0 + 1) * P]

        # ---- normalize+transpose into per-quad bf16 featT tiles ----
        fts = []
        for q in range(nquads):
            s0, s1 = q * spq, min((q + 1) * spq, nslots)
            wq = (s1 - s0) * slot
            pt = ptp.tile([P, wq], F32, tag="pt")
            for k in range(wq // P):
                t = s0 * tps + k
                nc.tensor.matmul(pt[:, k * P : (k + 1) * P],
                                 nat_tile(t), diag_tile(t),
                                 start=True, stop=True)
            ft = singles.tile([P, wq], BF16, tag=f"ft{q}")
            nc.vector.tensor_copy(ft, pt)
            fts.append(ft)

        def ft_slice(s, a, b):
            return fts[s // spq][:, (s % spq) * slot + a : (s % spq) * slot + b]

        # ---- per-pair Gram PSUM + direct sqrt + weighted row-reduce ----
        acc = singles.tile([P, nslots], F32)
        wscr = singles.tile([P, tps * slot], BF16)
        for p in range(npairs):
            s0, s1 = 2 * p, min(2 * p + 2, nslots)
            wp = (s1 - s0) * tps * slot
            gt = gp.tile([P, wp], F32, tag="gt")
            # per region: diag knockout (const operands, absorbs the PSUM WAR
            # wait on its first use) -> mask -> Gram, closing each
            # accumulation group before the next opens
            for s in range(s0, s1):
                for h in range(tps):
                    reg = (s - s0) * tps * slot + h * slot
                    off = (tps - 1 - h) * P
                    jc = slice(s * slot + h * P, s * slot + (h + 1) * P)
                    ic = slice(s * slot, (s + 1) * slot)
                    nc.tensor.matmul(gt[:, reg : reg + slot],
                                     ident, idn3[:, off : off + slot],
                                     start=True, stop=False)
                    nc.tensor.matmul(gt[:, reg : reg + slot],
                                     mneg[:, jc], mpos[:, ic],
                                     start=False, stop=False)
                    nc.tensor.matmul(gt[:, reg : reg + slot],
                                     ft_slice(s, h * P, (h + 1) * P),
                                     ft_slice(s, 0, slot),
                                     start=False, stop=True)
            dq = singles.tile([P, wp], BF16, tag=f"dq{p}")
            nc.scalar.activation(dq, gt, mybir.ActivationFunctionType.Sqrt,
                                 scale=-2.0)
            for s in range(s0, s1):
                o = (s - s0) * tps * slot
                nc.vector.tensor_scalar(
                    out=wscr, in0=dq[:, o : o + tps * slot],
                    scalar1=wb[:, s : s + 1], scalar2=None,
                    op0=mybir.AluOpType.mult, op1=mybir.AluOpType.add,
                    accum_out=acc[:, s : s + 1],
                )

        # ---- core partial = sum(acc) -> DRAM ----
        accsum = singles.tile([P, 1], F32)
        nc.vector.tensor_reduce(
            accsum, acc, axis=mybir.AxisListType.X, op=mybir.AluOpType.add,
        )
        colpsum = ptp.tile([1, 1], F32, tag="pt")
        nc.tensor.matmul(colpsum, ones, accsum, start=True, stop=True)
        partial = singles.tile([1, 1], F32)
        nc.vector.tensor_copy(partial, colpsum)
        nc.sync.dma_start(out=out_d[:, :], in_=partial)

    nc.compile()
    return nc


def _shard_inputs(features: np.ndarray, labels: np.ndarray):
    """Sort rows by class, pad each class to a slot, deal slots to cores."""
    n = features.shape[0]
    classes, counts = np.unique(labels, return_counts=True)
    c = len(classes)
    nslots = -(-c // N_CORES)
    slot = max(256, -(-int(counts.max()) // P) * P)
    rows = nslots * slot

    order = np.argsort(labels, kind="stable")
    bounds = np.concatenate([[0], np.cumsum(counts)])

    in_maps = []
    for core in range(N_CORES):
        feat = np.zeros((rows, P), np.float32)
        mpos = np.zeros((1, rows), ml_dtypes.bfloat16)
        wrow = np.zeros((1, nslots), np.float32)
        for k in range(nslots):
            g = core * nslots + k
            if g >= c:
                continue
            cnt = int(counts[g])
            rows_g = order[bounds[g] : bounds[g + 1]]
            feat[k * slot : k * slot + cnt] = features[rows_g]
            mpos[0, k * slot : k * slot + cnt] = 1.0
            wrow[0, k] = 1.0 / (cnt * n)
        in_maps.append(
            {"feat": feat, "mpos": mpos, "mneg": -mpos, "wrow": wrow}
        )
    # host-side correction for the diagonal knockout mass
    corr = slot * np.sqrt(2.0 * DC) * float((1.0 / (counts.astype(np.float64) * n)).sum())
    return in_maps, nslots, slot, corr


def _run(features, labels, **spmd_kwargs):
    features = np.asarray(features, np.float32)
    labels = np.asarray(labels).reshape(-1)
    in_maps, nslots, slot, corr = _shard_inputs(features, labels)
    nc = _build_program(nslots, slot)
    res = run_bass_kernel_spmd(nc, in_maps, core_ids=list(range(N_CORES)),
                               **spmd_kwargs)
    total = 0.0
    for r in res.results:
        total += float(r["out"].reshape(-1)[0])
    return np.float32(total - corr), res


def kernel(features, labels):
    out, _ = _run(features, labels)
    return out


# revision 52
# speedup vs baseline: 1.2136x; 1.0506x over previous
"""EucNormLoss Trainium2 kernel (8-core SPMD).

loss = mean_i( sum_j d(i,j)*[l_i==l_j] / #{j: l_j==l_i} ),
d(i,j) = sqrt(relu(2 - 2*fn_i.fn_j)) on L2-normalized rows.

Only same-class pairs matter, so the host sorts rows by class (pure data
movement), pads each class to a fixed `slot` of rows, and deals an equal
number of class slots to each core.  Per slot-pair, PSUM accumulates
  psum = g - m_j*m_i - Dc*delta_ij
via bf16 matmuls (Gram + rank-1 validity mask + diagonal knockout), where
Dc = 2^-9 keeps -psum >= 0 everywhere: valid pairs give 1-g >= ~0.4, pad
pairs give exactly 0, diagonals give Dc + (1 - g_ii) > 0.  ACT then takes
sqrt(-psum) straight out of PSUM (no relu pass), and a bf16 4x-mode
tensor_scalar row-reduces each slot with weight 1/(n_c*N) via accum_out.
The known diagonal mass slot*sqrt(Dc)*sum_c 1/(n_c*N) is subtracted on
the host (its data-dependent residual is O(1e-6) relative).

Normalization happens on-device: per-row 1/max(||x||,1e-12) is folded
into the transpose as featT_tile = nat_tile.T @ diag(rinv), one fp32
matmul per 128-row tile; the PSUM->SBUF copy casts to bf16.

Structure is chunked so everything pipelines: 7 input-DMA chunks (issued
from both SP and ACT queues), per-chunk sumsq/norms/diag, per-quad featT
tiles, per-pair Gram PSUM.  Separate SBUF tiles per stage avoid WAR waits
(Tile tracks deps per tile; most ISA structs encode a single sync wait,
matmuls get a second via Bacc's move_matmul_waits_to_ldweights).
"""

import sys

import numpy as np

for _p in ("/opt/trn_rl_repo",):
    if _p not in sys.path:
        sys.path.insert(0, _p)

import ml_dtypes
from contextlib import ExitStack

import concourse.bass as bass
import concourse.bacc as bacc
import concourse.tile as tile
from concourse import mybir
from concourse.bass_utils import run_bass_kernel_spmd
from concourse.masks import make_identity

N_CORES = 8
P = 128          # partitions / feature dim
DC = 2.0 ** -6   # diagonal knockout; must exceed bf16 rounding of g_ii (half-ulp 2^-8)

F32 = mybir.dt.float32
BF16 = mybir.dt.bfloat16


def _bcast_rows(ap: bass.AP, n: int) -> bass.AP:
    """Broadcast a [1, ...] AP across n partitions (partition step 0)."""
    return bass.AP(tensor=ap.tensor, offset=ap.offset,
                   ap=[[0, n]] + list(ap.ap[1:]))


def _build_program(nslots: int, slot: int):
    rows = nslots * slot
    nt = rows // P               # 128-row tiles
    tps = slot // P              # tiles per slot
    spq = 2                      # slots per featT tile (aligns with pairs/chunks)
    nquads = -(-nslots // spq)
    npairs = -(-nslots // 2)

    nc = bacc.Bacc(None, target_bir_lowering=False)
    feat_d = nc.declare_dram_parameter("feat", [rows, P], F32, isOutput=False)
    mpos_d = nc.declare_dram_parameter("mpos", [1, rows], BF16, isOutput=False)
    mneg_d = nc.declare_dram_parameter("mneg", [1, rows], BF16, isOutput=False)
    wrow_d = nc.declare_dram_parameter("wrow", [1, nslots], F32, isOutput=False)
    out_d = nc.declare_dram_parameter("out", [1, 1], F32, isOutput=True)

    with ExitStack() as ctx:
        tc = ctx.enter_context(tile.TileContext(nc))
        consts = ctx.enter_context(tc.tile_pool(name="consts", bufs=1))
        singles = ctx.enter_context(tc.tile_pool(name="singles", bufs=1))
        ptp = ctx.enter_context(tc.tile_pool(name="ptp", bufs=2, space="PSUM"))
        gp = ctx.enter_context(tc.tile_pool(name="gp", bufs=3, space="PSUM"))

        # consts: ident (1.0 I), idn3 = [0 | -Dc*I | 0] for diag knockout
        ident = consts.tile([P, P], BF16)
        make_identity(nc, ident)
        idn3 = consts.tile([P, 3 * P], BF16)
        nc.gpsimd.memset(idn3, 0.0)
        nc.gpsimd.affine_select(
            out=idn3[:, P : 2 * P], in_=idn3[:, P : 2 * P],
            compare_op=mybir.AluOpType.not_equal,
            fill=-DC, base=0, pattern=[[-1, P]], channel_multiplier=1,
        )
        ones = consts.tile([P, 1], F32)
        nc.vector.memset(ones, 1.0)
        onesb = consts.tile([P, 1], BF16)
        nc.vector.memset(onesb, 1.0)
        # tiny bias inside sqrt(ss): pad rows get norm 1e-10 instead of 0,
        # so the reciprocal stays finite without a separate max() op
        nepsb = consts.tile([P, 1], F32)
        nc.vector.memset(nepsb, 1e-20)

        mpos = singles.tile([1, rows], BF16)
        mneg = singles.tile([1, rows], BF16)
        wb = singles.tile([P, nslots], F32)
        nc.sync.dma_start(out=mpos, in_=mpos_d[:, :])
        nc.sync.dma_start(out=mneg, in_=mneg_d[:, :])
        nc.sync.dma_start(out=wb, in_=_bcast_rows(wrow_d[:, :], P))

        # ---- chunked load + per-row sumsq + norms + diag(rinv) ----
        chunk = 4                                    # tiles per chunk
        cbounds = list(range(0, nt, chunk)) + [nt]
        nch = len(cbounds) - 1
        nats, diags, rinvs = [], [], []
        for c in range(nch):
            t0, t1 = cbounds[c], cbounds[c + 1]
            w = (t1 - t0) * P
            natc = singles.tile([P, w], F32, tag=f"nat{c}")
            src = feat_d[t0 * P : t1 * P, :]
            src3 = bass.AP(tensor=src.tensor, offset=src.offset,
                           ap=[[P, P], [P * P, t1 - t0], [1, P]])
            eng = nc.sync if c % 2 == 0 else nc.scalar
            eng.dma_start(out=natc[:, :].rearrange("p (t d) -> p t d", d=P),
                          in_=src3)
            sq = singles.tile([P, w], F32, tag=f"sq{c}")
            if c % 2 == 0:
                nc.gpsimd.tensor_mul(sq, natc, natc)
            else:
                nc.vector.tensor_mul(sq, natc, natc)
            ss = singles.tile([P, t1 - t0], F32, tag=f"ss{c}")
            nc.vector.tensor_reduce(
                ss, sq[:, :].rearrange("p (t d) -> p t d", d=P),
                axis=mybir.AxisListType.X, op=mybir.AluOpType.add,
            )
            nrm = singles.tile([P, t1 - t0], F32, tag=f"nr{c}")
            nc.scalar.activation(nrm, ss, mybir.ActivationFunctionType.Sqrt,
                                 bias=nepsb[:, 0:1])
            rinv = singles.tile([P, t1 - t0], F32, tag=f"ri{c}")
            nc.vector.reciprocal(rinv, nrm)
            dg = singles.tile([P, w], F32, tag=f"dg{c}")
            rb = bass.AP(tensor=rinv[:, :].tensor, offset=rinv[:, :].offset,
                         ap=list(rinv[:, :].ap) + [[0, P]])
            nc.gpsimd.affine_select(
                out=dg[:, :].rearrange("p (t d) -> p t d", d=P), in_=rb,
                compare_op=mybir.AluOpType.is_equal, fill=0.0, base=0,
                pattern=[[0, t1 - t0], [-1, P]], channel_multiplier=1,
            )
            nats.append((t0, natc))
            diags.append(dg)
            rinvs.append(rinv)

        def nat_tile(t):
            c = t // chunk
            t0, natc = nats[c]
            return natc[:, (t - t0) * P : (t - t0 + 1) * P]

        def diag_tile(t):
            c = t // chunk
            t0, _ = nats[c]
            return diags[c][:, (t - t0) * P : (t - t0 + 1) * P]

        # ---- normalize+transpose into per-quad bf16 featT tiles ----
        fts = []
        for q in range(nquads):
            s0, s1 = q * spq, min((q + 1) * spq, nslots)
            wq = (s1 - s0) * slot
            pt = ptp.tile([P, wq], F32, tag="pt")
            for k in range(wq // P):
                t = s0 * tps + k
                nc.tensor.matmul(pt[:, k * P : (k + 1) * P],
                                 nat_tile(t), diag_tile(t),
                                 start=True, stop=True)
            ft = singles.tile([P, wq], BF16, tag=f"ft{q}")
            nc.vector.tensor_copy(ft, pt)
            fts.append(ft)

        def ft_slice(s, a, b):
            return fts[s // spq][:, (s % spq) * slot + a : (s % spq) * slot + b]

        # ---- per-pair Gram PSUM + direct sqrt + weighted row-reduce ----
        acc = singles.tile([P, nslots], F32)
        wscr = singles.tile([P, tps * slot], BF16)
        for p in range(npairs):
            s0, s1 = 2 * p, min(2 * p + 2, nslots)
            wp = (s1 - s0) * tps * slot
            gt = gp.tile([P, wp], F32, tag="gt")
            # per region: diag knockout (const operands, absorbs the PSUM WAR
            # wait on its first use) -> mask -> Gram, closing each
            # accumulation group before the next opens
            for s in range(s0, s1):
                for h in range(tps):
                    reg = (s - s0) * tps * slot + h * slot
                    off = (tps - 1 - h) * P
                    jc = slice(s * slot + h * P, s * slot + (h + 1) * P)
                    ic = slice(s * slot, (s + 1) * slot)
                    nc.tensor.matmul(gt[:, reg : reg + slot],
                                     ident, idn3[:, off : off + slot],
                                     start=True, stop=False)
                    nc.tensor.matmul(gt[:, reg : reg + slot],
                                     mneg[:, jc], mpos[:, ic],
                                     start=False, stop=False)
                    nc.tensor.matmul(gt[:, reg : reg + slot],
                                     ft_slice(s, h * P, (h + 1) * P),
                                     ft_slice(s, 0, slot),
                                     start=False, stop=True)
            dq = singles.tile([P, wp], BF16, tag=f"dq{p}")
            nc.scalar.activation(dq, gt, mybir.ActivationFunctionType.Sqrt,
                                 scale=-2.0)
            for s in range(s0, s1):
                o = (s - s0) * tps * slot
                nc.vector.tensor_scalar(
                    out=wscr, in0=dq[:, o : o + tps * slot],
                    scalar1=wb[:, s : s + 1], scalar2=None,
                    op0=mybir.AluOpType.mult, op1=mybir.AluOpType.add,
                    accum_out=acc[:, s : s + 1],
                )

        # ---- core partial = sum(acc) -> DRAM ----
        accsum = singles.tile([P, 1], F32)
        nc.vector.tensor_reduce(
            accsum, acc, axis=mybir.AxisListType.X, op=mybir.AluOpType.add,
        )
        colpsum = ptp.tile([1, 1], F32, tag="pt")
        nc.tensor.matmul(colpsum, ones, accsum, start=True, stop=True)
        partial = singles.tile([1, 1], F32)
        nc.vector.tensor_copy(partial, colpsum)
        nc.sync.dma_start(out=out_d[:, :], in_=partial)

    nc.compile()
    return nc


def _shard_inputs(features: np.ndarray, labels: np.ndarray):
    """Sort rows by class, pad each class to a slot, deal slots to cores."""
    n = features.shape[0]
    classes, counts = np.unique(labels, return_counts=True)
    c = len(classes)
    nslots = -(-c // N_CORES)
    slot = max(256, -(-int(counts.max()) // P) * P)
    rows = nslots * slot

    order = np.argsort(labels, kind="stable")
    bounds = np.concatenate([[0], np.cumsum(counts)])

    in_maps = []
    for core in range(N_CORES):
        feat = np.zeros((rows, P), np.float32)
        mpos = np.zeros((1, rows), ml_dtypes.bfloat16)
        wrow = np.zeros((1, nslots), np.float32)
        for k in range(nslots):
            g = core * nslots + k
            if g >= c:
                continue
            cnt = int(counts[g])
            rows_g = order[bounds[g] : bounds[g + 1]]
            feat[k * slot : k * slot + cnt] = features[rows_g]
            mpos[0, k * slot : k * slot + cnt] = 1.0
            wrow[0, k] = 1.0 / (cnt * n)
        in_maps.append(
            {"feat": feat, "mpos": mpos, "mneg": -mpos, "wrow": wrow}
        )
    # host-side correction for the diagonal knockout mass
    corr = slot * np.sqrt(2.0 * DC) * float((1.0 / (counts.astype(np.float64) * n)).sum())
    return in_maps, nslots, slot, corr


def _run(features, labels, **spmd_kwargs):
    features = np.asarray(features, np.float32)
    labels = np.asarray(labels).reshape(-1)
    in_maps, nslots, slot, corr = _shard_inputs(features, labels)
    nc = _build_program(nslots, slot)
    res = run_bass_kernel_spmd(nc, in_maps, core_ids=list(range(N_CORES)),
                               **spmd_kwargs)
    total = 0.0
    for r in res.results:
        total += float(r["out"].reshape(-1)[0])
    return np.float32(total - corr), res


def kernel(features, labels):
    out, _ = _run(features, labels)
    return out
